# revision 24
# baseline (speedup 1.0000x reference)
"""Mixtral GQA attention (B=2, S=2048, H=4096, 32 q heads / 8 kv heads,
interleaved RoPE, causal; sliding window 4096 >= S so it is plain causal)
on 8 Trainium2 NeuronCores.

Sharding: DP=2 over batch x TP=4 over kv-head pairs. Core c = 4*b + t
handles batch b, kv heads {2t, 2t+1}, q heads [8t, 8t+8). Each core
computes qkv projection (transposed layout), RoPE, attention, and its
partial of the wo projection; the host sums the 4 partials per batch.

Perf notes (v4):
 - Everything computed transposed ([feature, token]); contraction on
   partitions. The q/k path runs in fp16 (scores err ~1e-3: fine), with
   the softmax 1/sqrt(d) folded into the exp activation's scale so the
   unscaled q weights stay clear of fp16 subnormals. probs / V / attn
   out / wo are bf16 (exp needs bf16 range). fp32 PSUM accumulation
   everywhere. fp16 halves LdWeights time, SBUF footprint, and DMA.
 - Stage 1 uses 2 H-segments -> 16-matmul PSUM accumulation groups
   (per-group overheads amortize; measured bank-switch cost ~95ns).
 - RoPE and V transposes interleave chunk-wise into the last H-segment
   (k/v feature blocks computed first), so the PE flows straight from
   projection into attention. RoPE runs on fp16 tiles at the DVE's
   2-byte fast mode; rotate-half copies go to the scalar engine.
 - Attention PE emission per 4-key-block batch is [4 scores][4 pv]
   [4 sum] (psum-bank switches amortize), software-pipelined one batch
   ahead across (head, chunk) boundaries to hide exp/mask latency.
 - Causal masking via gpsimd affine_select (keeps the DVE queue free:
   its head-of-line blocking caused stalls at attention entry/exit).
 - Softmax denominators: ones-column matmul; reciprocal_approx_fast;
   gpsimd partition_broadcast; final normalization mul on DVE delayed
   two batches so it never blocks ahead of mask work. attn output is
   one tile per (head, chunk) so stage 4's deps are chunk-granular.
 - h-tile DMAs ride the gpsimd SWDGE queue: on the sync queue they sat
   behind WAR-gated next-segment weight loads (head-of-line blocking).
"""

import sys

sys.path.insert(0, "/opt/trn_rl_repo")

import numpy as np
import ml_dtypes

import concourse.bass as bass  # noqa: F401
import concourse.mybir as mybir
import concourse.tile as tile
from concourse import bacc
from concourse.bass_utils import run_bass_kernel_spmd

F32 = mybir.dt.float32
F32R = mybir.dt.float32r
BF16 = mybir.dt.bfloat16
F16 = mybir.dt.float16

B = 2
S = 2048
H = 4096
NH = 32
NKV = 8
HD = 128
GROUP = NH // NKV
ROPE_BASE = 10000.0
SCALE = HD**-0.5

N_CORES = 8
TP = 4  # kv-head-pair groups
Q_PER_CORE = 8
KV_PER_CORE = 2

NC_BLK = Q_PER_CORE + 2 * KV_PER_CORE  # 12 feature blocks of 128 in stage 1
NSEG = 2  # contraction (H) segments
HB = H // 128 // NSEG  # h-blocks per segment = 16
TCH = 4  # token chunks
TC_W = S // TCH  # 512
SB = S // 128  # 16 key blocks

# stage-1 c-block order: k heads, v heads, then q heads (so rope-k and
# V transposes start as early as possible inside the last segment)
C_ORDER = [Q_PER_CORE, Q_PER_CORE + 1, Q_PER_CORE + 2, Q_PER_CORE + 3] + list(
    range(Q_PER_CORE)
)

_compiled = None


def _build():
    nc = bacc.Bacc("TRN2", target_bir_lowering=False, debug=False,
                   num_devices=N_CORES)

    hid_t = nc.declare_dram_parameter("hid_t", [H, S], F16, isOutput=False)
    w12 = nc.declare_dram_parameter("w12", [H, NC_BLK * 128], F16, isOutput=False)
    wo = nc.declare_dram_parameter("wo", [Q_PER_CORE * 128, H], BF16,
                                   isOutput=False)
    cos2 = nc.declare_dram_parameter("cos2", [128, S], F16, isOutput=False)
    sinpm = nc.declare_dram_parameter("sinpm", [128, S], F16, isOutput=False)
    identd = nc.declare_dram_parameter("identd", [128, 128], F16, isOutput=False)
    onescd = nc.declare_dram_parameter("onescd", [128, 1], BF16, isOutput=False)
    out = nc.declare_dram_parameter("out", [S, H], F32, isOutput=True)

    with tile.TileContext(nc) as tc:
        with tc.tile_pool(name="consts", bufs=1) as consts, \
             tc.tile_pool(name="acc", bufs=1) as accp:
            ident = consts.tile([128, 128], F16, name="ident", tag="ident")
            ones_c = consts.tile([128, 1], BF16, name="ones_c", tag="ones_c")
            cost = consts.tile([128, S], F16, name="cost", tag="cost")
            sint = consts.tile([128, S], F16, name="sint", tag="sint")
            warm = consts.tile([128, 1], F16, name="warm", tag="warm")

            # warm the Exp activation table long before attention needs it
            nc.sync.dma_start(out=warm[:], in_=identd[:, 0:1])
            nc.scalar.activation(warm[:], warm[:],
                                 mybir.ActivationFunctionType.Exp)

            # q/k accumulators in fp16 (scores precision ~1e-3, plenty)
            acc = [accp.tile([128, S], F16, name=f"acc{c}", tag=f"acc{c}")
                   for c in range(Q_PER_CORE + KV_PER_CORE)]

            vnat = [None] * (KV_PER_CORE * SB)

            # ---- stage 1: qkv^T = w12^T @ hid_t over 2 H-segments.
            # In the last segment, rope + V transposes interleave chunk-wise.
            with tc.tile_pool(name="accv", bufs=1) as accvp, \
                 tc.tile_pool(name="wseg", bufs=16) as wp, \
                 tc.tile_pool(name="hidt", bufs=32) as hp, \
                 tc.tile_pool(name="ropet", bufs=3) as rtp, \
                 tc.tile_pool(name="vnatp", bufs=1) as vp, \
                 tc.tile_pool(name="ps1", bufs=3, space="PSUM") as ps1, \
                 tc.tile_pool(name="ps2", bufs=2, space="PSUM") as ps2:

                accv = [accvp.tile([128, S], F16, name=f"accv{m}",
                                   tag=f"accv{m}") for m in range(KV_PER_CORE)]

                def acc_of(c):
                    if c < Q_PER_CORE + KV_PER_CORE:
                        return acc[c]
                    return accv[c - Q_PER_CORE - KV_PER_CORE]

                def rope_chunk(c, t):
                    """acc[c][:, chunk t] = acc*cos + rot_half(acc)*sin."""
                    lo, hi = t * TC_W, (t + 1) * TC_W
                    blk = acc[c][:, lo:hi]
                    tmp = rtp.tile([128, TC_W], F16, name=f"rt{c}_{t}",
                                   tag="ropetmp")
                    # rotate-half copies on the scalar engine (act is idle
                    # during stage 1); muls/add all-fp16 -> DVE fast mode
                    nc.scalar.copy(tmp[0:64, :], blk[64:128, :])
                    nc.scalar.copy(tmp[64:128, :], blk[0:64, :])
                    nc.vector.tensor_mul(tmp[:], tmp[:], sint[:, lo:hi])
                    nc.vector.tensor_mul(blk, blk, cost[:, lo:hi])
                    nc.vector.tensor_add(blk, blk, tmp[:])

                for seg in range(NSEG):
                    last = seg == NSEG - 1
                    wt = {}
                    for c in C_ORDER:
                        w_tile = wp.tile([128, HB, 128], F16,
                                         name=f"w_{seg}_{c}", tag="w")
                        nc.sync.dma_start(
                            out=w_tile[:],
                            in_=w12[seg * HB * 128:(seg + 1) * HB * 128,
                                    c * 128:(c + 1) * 128]
                            .rearrange("(hb p) c -> p hb c", p=128),
                        )
                        wt[c] = w_tile
                    if seg == 1:
                        # constants are only needed from the last segment
                        # on; load them behind seg 0's weight+hid DMAs.
                        nc.sync.dma_start(out=cost[:], in_=cos2[:])
                        nc.sync.dma_start(out=sint[:], in_=sinpm[:])
                        nc.sync.dma_start(out=ident[:], in_=identd[:])
                        nc.sync.dma_start(out=ones_c[:], in_=onescd[:])
                    for t in range(TCH):
                        ht = []
                        for hb in range(HB):
                            h_tile = hp.tile([128, TC_W], F16,
                                             name=f"h_{seg}_{t}_{hb}",
                                             tag="h")
                            # gpsimd (SWDGE) queue: h loads must not sit
                            # behind next-seg w loads that are WAR-gated
                            # on this seg's compute (head-of-line block)
                            nc.gpsimd.dma_start(
                                out=h_tile[:],
                                in_=hid_t[(seg * HB + hb) * 128:
                                          (seg * HB + hb + 1) * 128,
                                          t * TC_W:(t + 1) * TC_W],
                            )
                            ht.append(h_tile)
                        for ci, c in enumerate(C_ORDER):
                            pt = ps1.tile([128, TC_W], F32,
                                          name=f"p1_{seg}_{t}_{c}", tag="ps1")
                            for hb in range(HB):
                                nc.tensor.matmul(pt[:], wt[c][:, hb, :],
                                                 ht[hb][:],
                                                 start=(hb == 0),
                                                 stop=(hb == HB - 1))
                            dst = acc_of(c)[:, t * TC_W:(t + 1) * TC_W]
                            if seg == 0:
                                nc.vector.tensor_copy(dst, pt[:])
                            else:
                                nc.vector.tensor_add(dst, dst, pt[:])
                            if last:
                                if c < Q_PER_CORE + KV_PER_CORE:
                                    rope_chunk(c, t)
                                if ci == 5:
                                    # V transposes for this chunk's 4 key
                                    # blocks; emitted a few psum groups after
                                    # the v drains so the DVE is surely ahead.
                                    for kv in range(KV_PER_CORE):
                                        for sb in range(4 * t, 4 * t + 4):
                                            ptt = ps2.tile(
                                                [128, 128], F16,
                                                name=f"pt2_{kv}_{sb}",
                                                tag="ps2")
                                            nc.tensor.transpose(
                                                ptt[:],
                                                accv[kv][:, sb * 128:
                                                         (sb + 1) * 128],
                                                ident[:],
                                            )
                                            vtile = vp.tile(
                                                [128, 128], BF16,
                                                name=f"v{kv}_{sb}",
                                                tag=f"v{kv}_{sb}")
                                            nc.scalar.copy(vtile[:], ptt[:])
                                            vnat[kv * SB + sb] = vtile

            # ---- stage 3 + 4 share the attention-output accumulator.
            # One tile per (head, chunk): keeps stage-4 LdWeights deps at
            # chunk granularity so stage 4 can start before the s3 tail.
            with tc.tile_pool(name="acco", bufs=1) as accop:
                acco = [[accop.tile([128, TC_W], BF16, name=f"acco{g}_{t}",
                                    tag=f"acco{g}_{t}") for t in range(TCH)]
                        for g in range(Q_PER_CORE)]

                # ---- stage 3: attention; batched emission, 1-batch pipeline
                with tc.tile_pool(name="probs", bufs=4) as pp, \
                     tc.tile_pool(name="recip", bufs=2) as rcp, \
                     tc.tile_pool(name="rbc", bufs=2) as rbp, \
                     tc.tile_pool(name="ps_s", bufs=2, space="PSUM") as ps_s, \
                     tc.tile_pool(name="ps_pv", bufs=2, space="PSUM") as ps_pv, \
                     tc.tile_pool(name="ps_sum", bufs=2, space="PSUM") as ps_sm:

                    # batches of 4 key blocks; PE emission per batch is
                    # [4 scores fp16] [4 pv bf16] [4 sum bf16] so psum-bank
                    # switches (and their per-group overhead) amortize.
                    # The diagonal batch (bb == t, the one whose exp->select
                    # chain is longest) goes FIRST within each (g, t) so the
                    # s3 tail never waits on serialized gpsimd selects.
                    batches = [(g, t, bb, ei)
                               for g in range(Q_PER_CORE)
                               for t in range(TCH)
                               for ei, bb in enumerate([t] + list(range(t)))]
                    prs = {}

                    def emit_batch(bi):
                        g, t, bb, _ei = batches[bi]
                        kv = g // GROUP
                        kt = acc[Q_PER_CORE + kv]
                        prl = []
                        for p in range(2):
                            # 2-bank psum pair: two score matmuls, ONE wide
                            # exp (halves the activation-engine overhead)
                            sc = ps_s.tile([128, 2, TC_W], F32,
                                           name=f"sc_{g}_{t}_{bb}_{p}",
                                           tag="s")
                            for i in range(2):
                                sb = 4 * bb + 2 * p + i
                                nc.tensor.matmul(
                                    sc[:, i, :],
                                    kt[:, sb * 128:(sb + 1) * 128],
                                    acc[g][:, t * TC_W:(t + 1) * TC_W],
                                    start=True, stop=True,
                                )
                            pr = pp.tile([128, 2, TC_W], BF16,
                                         name=f"pr_{g}_{t}_{bb}_{p}",
                                         tag="pr")
                            # softmax scale folded into the activation
                            nc.scalar.activation(
                                pr[:], sc[:], mybir.ActivationFunctionType.Exp,
                                scale=float(SCALE))
                            for i in range(2):
                                sb = 4 * bb + 2 * p + i
                                jd = sb - 4 * t
                                if jd >= 0:
                                    # zero where key > query; gpsimd keeps
                                    # the DVE queue free for recip/final-mul
                                    nc.gpsimd.affine_select(
                                        out=pr[:, i, :], in_=pr[:, i, :],
                                        compare_op=mybir.AluOpType.is_ge,
                                        fill=0.0, base=-128 * jd,
                                        pattern=[[1, TC_W]],
                                        channel_multiplier=-1,
                                    )
                                prl.append(pr[:, i, :])
                        prs[bi] = prl

                    emit_batch(0)
                    pv = None
                    sm = None
                    pending = {}  # bi -> (g, t, pv, rcb) final-mul to emit
                    for bi, (g, t, bb, ei) in enumerate(batches):
                        if bi in pending:
                            # delayed normalization mul: broadcast has landed
                            # by now, so this never blocks the DVE queue
                            pg, pt, ppv, prcb = pending.pop(bi)
                            nc.vector.tensor_mul(acco[pg][pt][:], ppv[:],
                                                 prcb[:])
                        if bi + 1 < len(batches):
                            emit_batch(bi + 1)
                        kv = g // GROUP
                        if ei == 0:
                            pv = ps_pv.tile([128, TC_W], F32,
                                            name=f"pv_{g}_{t}", tag="pv")
                            sm = ps_sm.tile([1, TC_W], F32,
                                            name=f"sm_{g}_{t}", tag="sum")
                        prl = prs.pop(bi)
                        for i in range(4):
                            sb = 4 * bb + i
                            first = ei == 0 and i == 0
                            lastm = ei == t and i == 3
                            nc.tensor.matmul(pv[:], vnat[kv * SB + sb][:],
                                             prl[i], start=first,
                                             stop=lastm)
                        for i in range(4):
                            sb = 4 * bb + i
                            first = ei == 0 and i == 0
                            lastm = ei == t and i == 3
                            nc.tensor.matmul(sm[:], ones_c[:], prl[i],
                                             start=first,
                                             stop=lastm)
                        if ei == t:
                            # normalization chain, fully off the PE
                            rc = rcp.tile([1, TC_W], F32,
                                          name=f"rc_{g}_{t}", tag="rc")
                            nc.vector.reciprocal_approx_fast(rc[:], sm[:])
                            rcb = rbp.tile([128, TC_W], F32,
                                           name=f"rcb_{g}_{t}", tag="rcb")
                            nc.gpsimd.partition_broadcast(rcb[:], rc[:])
                            pending[bi + 2] = (g, t, pv, rcb)
                    for pg, pt, ppv, prcb in pending.values():
                        nc.vector.tensor_mul(acco[pg][pt][:], ppv[:], prcb[:])

                # ---- stage 4: out[t, n] = sum_g attn_g^T @ wo_g  (bf16)
                with tc.tile_pool(name="wop", bufs=8) as wops, \
                     tc.tile_pool(name="outp", bufs=4) as op, \
                     tc.tile_pool(name="ps4", bufs=4, space="PSUM") as ps4:
                    for n in range(H // TC_W):
                        wq4 = []
                        for q4 in range(4):
                            wn = wops.tile([128, 2, TC_W], BF16,
                                           name=f"wo_{n}_{q4}", tag="wo")
                            nc.sync.dma_start(
                                out=wn[:],
                                in_=wo[q4 * 256:(q4 + 1) * 256,
                                       n * TC_W:(n + 1) * TC_W]
                                .rearrange("(g p) c -> p g c", p=128),
                            )
                            wq4.append(wn)
                        for tb in range(SB):
                            po = ps4.tile([128, TC_W], F32,
                                          name=f"po_{n}_{tb}", tag="po")
                            for g in range(Q_PER_CORE):
                                nc.tensor.matmul(
                                    po[:],
                                    acco[g][tb // 4][:, (tb % 4) * 128:
                                                     (tb % 4 + 1) * 128],
                                    wq4[g // 2][:, g % 2, :],
                                    start=(g == 0),
                                    stop=(g == Q_PER_CORE - 1),
                                )
                            ot = op.tile([128, TC_W], F32,
                                         name=f"ot_{n}_{tb}", tag="ot")
                            nc.scalar.copy(ot[:], po[:])
                            nc.gpsimd.dma_start(
                                out=out[tb * 128:(tb + 1) * 128,
                                        n * TC_W:(n + 1) * TC_W],
                                in_=ot[:],
                            )

    nc.compile()
    return nc


def _get_compiled():
    global _compiled
    if _compiled is None:
        _compiled = _build()
    return _compiled


_EVEN_ODD = np.concatenate([np.arange(0, HD, 2), np.arange(1, HD, 2)])


def _prep_core_inputs(hidden_states, positions, wqkv, wo):
    """Returns list of 8 in_maps (core c = 4*b + t)."""
    inv_freq = ROPE_BASE ** (-np.arange(0, HD, 2, dtype=np.float32) / HD)
    ident = np.eye(128, dtype=np.float16)
    ones_c = np.ones((128, 1), dtype=ml_dtypes.bfloat16)

    per_batch = []
    for b in range(B):
        hid_t = np.ascontiguousarray(hidden_states[b].T.astype(np.float16))
        ang = positions[b].astype(np.float32)[:, None] * inv_freq[None, :]
        cos = np.cos(ang).T.astype(np.float32)  # [64, S]
        sin = np.sin(ang).T.astype(np.float32)
        cos2 = np.ascontiguousarray(
            np.concatenate([cos, cos], axis=0).astype(np.float16))
        sinpm = np.ascontiguousarray(
            np.concatenate([-sin, sin], axis=0).astype(np.float16))
        per_batch.append((hid_t, cos2, sinpm))

    in_maps = []
    for c in range(N_CORES):
        b, t = c // TP, c % TP
        hid_t, cos2, sinpm = per_batch[b]
        blocks = []
        for gh in range(Q_PER_CORE):  # q heads, permuted (softmax scale is
            h = Q_PER_CORE * t + gh   # applied in the exp activation)
            blocks.append(wqkv[:, h * HD:(h + 1) * HD][:, _EVEN_ODD])
        for m in range(KV_PER_CORE):  # k heads, permuted
            h = KV_PER_CORE * t + m
            blocks.append(
                wqkv[:, NH * HD + h * HD: NH * HD + (h + 1) * HD][:, _EVEN_ODD])
        for m in range(KV_PER_CORE):  # v heads, natural
            h = KV_PER_CORE * t + m
            base = (NH + NKV) * HD
            blocks.append(wqkv[:, base + h * HD: base + (h + 1) * HD])
        w12 = np.ascontiguousarray(
            np.concatenate(blocks, axis=1).astype(np.float16))
        wo_shard = np.ascontiguousarray(
            wo[Q_PER_CORE * HD * t: Q_PER_CORE * HD * (t + 1), :]
            .astype(ml_dtypes.bfloat16))
        in_maps.append({
            "hid_t": hid_t, "w12": w12, "wo": wo_shard,
            "cos2": cos2, "sinpm": sinpm,
            "identd": ident, "onescd": ones_c,
        })
    return in_maps


def kernel(hidden_states, positions, wqkv, wo):
    hidden_states = np.asarray(hidden_states)
    positions = np.asarray(positions)
    wqkv = np.asarray(wqkv)
    wo = np.asarray(wo)
    nc = _get_compiled()
    in_maps = _prep_core_inputs(hidden_states, positions, wqkv, wo)
    res = run_bass_kernel_spmd(nc, in_maps, list(range(N_CORES)))
    full = np.zeros((B, S, H), dtype=np.float32)
    for c in range(N_CORES):
        full[c // TP] += res.results[c]["out"]
    return full


# revision 25
# speedup vs baseline: 1.0005x; 1.0005x over previous
"""Mixtral GQA attention (B=2, S=2048, H=4096, 32 q heads / 8 kv heads,
interleaved RoPE, causal; sliding window 4096 >= S so it is plain causal)
on 8 Trainium2 NeuronCores.

Sharding: DP=2 over batch x TP=4 over kv-head pairs. Core c = 4*b + t
handles batch b, kv heads {2t, 2t+1}, q heads [8t, 8t+8). Each core
computes qkv projection (transposed layout), RoPE, attention, and its
partial of the wo projection; the host sums the 4 partials per batch.

Perf notes (v4):
 - Everything computed transposed ([feature, token]); contraction on
   partitions. The q/k path runs in fp16 (scores err ~1e-3: fine), with
   the softmax 1/sqrt(d) folded into the exp activation's scale so the
   unscaled q weights stay clear of fp16 subnormals. probs / V / attn
   out / wo are bf16 (exp needs bf16 range). fp32 PSUM accumulation
   everywhere. fp16 halves LdWeights time, SBUF footprint, and DMA.
 - Stage 1 uses 2 H-segments -> 16-matmul PSUM accumulation groups
   (per-group overheads amortize; measured bank-switch cost ~95ns).
 - RoPE and V transposes interleave chunk-wise into the last H-segment
   (k/v feature blocks computed first), so the PE flows straight from
   projection into attention. RoPE runs on fp16 tiles at the DVE's
   2-byte fast mode; rotate-half copies go to the scalar engine.
 - Attention PE emission per 4-key-block batch is [4 scores][4 pv]
   [4 sum] (psum-bank switches amortize), software-pipelined one batch
   ahead across (head, chunk) boundaries to hide exp/mask latency.
 - Causal masking via gpsimd affine_select (keeps the DVE queue free:
   its head-of-line blocking caused stalls at attention entry/exit).
 - Softmax denominators: ones-column matmul; reciprocal_approx_fast;
   gpsimd partition_broadcast; final normalization mul on DVE delayed
   two batches so it never blocks ahead of mask work. attn output is
   one tile per (head, chunk) so stage 4's deps are chunk-granular.
 - h-tile DMAs ride the gpsimd SWDGE queue: on the sync queue they sat
   behind WAR-gated next-segment weight loads (head-of-line blocking).
"""

import sys

sys.path.insert(0, "/opt/trn_rl_repo")

import numpy as np
import ml_dtypes

import concourse.bass as bass  # noqa: F401
import concourse.mybir as mybir
import concourse.tile as tile
from concourse import bacc
from concourse.bass_utils import run_bass_kernel_spmd

F32 = mybir.dt.float32
F32R = mybir.dt.float32r
BF16 = mybir.dt.bfloat16
F16 = mybir.dt.float16

B = 2
S = 2048
H = 4096
NH = 32
NKV = 8
HD = 128
GROUP = NH // NKV
ROPE_BASE = 10000.0
SCALE = HD**-0.5

N_CORES = 8
TP = 4  # kv-head-pair groups
Q_PER_CORE = 8
KV_PER_CORE = 2

NC_BLK = Q_PER_CORE + 2 * KV_PER_CORE  # 12 feature blocks of 128 in stage 1
NSEG = 2  # contraction (H) segments
HB = H // 128 // NSEG  # h-blocks per segment = 16
TCH = 4  # token chunks
TC_W = S // TCH  # 512
SB = S // 128  # 16 key blocks

# stage-1 c-block order: k heads, v heads, then q heads (so rope-k and
# V transposes start as early as possible inside the last segment)
C_ORDER = [Q_PER_CORE, Q_PER_CORE + 1, Q_PER_CORE + 2, Q_PER_CORE + 3] + list(
    range(Q_PER_CORE)
)

_compiled = None


def _build():
    nc = bacc.Bacc("TRN2", target_bir_lowering=False, debug=False,
                   num_devices=N_CORES)

    hid_t = nc.declare_dram_parameter("hid_t", [H, S], F16, isOutput=False)
    w12 = nc.declare_dram_parameter("w12", [H, NC_BLK * 128], F16, isOutput=False)
    wo = nc.declare_dram_parameter("wo", [Q_PER_CORE * 128, H], BF16,
                                   isOutput=False)
    cos2 = nc.declare_dram_parameter("cos2", [128, S], F16, isOutput=False)
    sinpm = nc.declare_dram_parameter("sinpm", [128, S], F16, isOutput=False)
    identd = nc.declare_dram_parameter("identd", [128, 128], F16, isOutput=False)
    onescd = nc.declare_dram_parameter("onescd", [128, 1], BF16, isOutput=False)
    out = nc.declare_dram_parameter("out", [S, H], F32, isOutput=True)

    with tile.TileContext(nc) as tc:
        with tc.tile_pool(name="consts", bufs=1) as consts, \
             tc.tile_pool(name="acc", bufs=1) as accp:
            ident = consts.tile([128, 128], F16, name="ident", tag="ident")
            ones_c = consts.tile([128, 1], BF16, name="ones_c", tag="ones_c")
            cost = consts.tile([128, S], F16, name="cost", tag="cost")
            sint = consts.tile([128, S], F16, name="sint", tag="sint")
            warm = consts.tile([128, 1], F16, name="warm", tag="warm")

            # warm the Exp activation table long before attention needs it
            nc.sync.dma_start(out=warm[:], in_=identd[:, 0:1])
            nc.scalar.activation(warm[:], warm[:],
                                 mybir.ActivationFunctionType.Exp)

            # q/k accumulators in fp16 (scores precision ~1e-3, plenty)
            acc = [accp.tile([128, S], F16, name=f"acc{c}", tag=f"acc{c}")
                   for c in range(Q_PER_CORE + KV_PER_CORE)]

            vnat = [None] * (KV_PER_CORE * SB)

            # ---- stage 1: qkv^T = w12^T @ hid_t over 2 H-segments.
            # In the last segment, rope + V transposes interleave chunk-wise.
            with tc.tile_pool(name="accv", bufs=1) as accvp, \
                 tc.tile_pool(name="wseg", bufs=16) as wp, \
                 tc.tile_pool(name="hidt", bufs=32) as hp, \
                 tc.tile_pool(name="ropet", bufs=3) as rtp, \
                 tc.tile_pool(name="vnatp", bufs=1) as vp, \
                 tc.tile_pool(name="ps1", bufs=3, space="PSUM") as ps1, \
                 tc.tile_pool(name="ps2", bufs=2, space="PSUM") as ps2:

                accv = [accvp.tile([128, S], F16, name=f"accv{m}",
                                   tag=f"accv{m}") for m in range(KV_PER_CORE)]

                def acc_of(c):
                    if c < Q_PER_CORE + KV_PER_CORE:
                        return acc[c]
                    return accv[c - Q_PER_CORE - KV_PER_CORE]

                def rope_chunk(c, t):
                    """acc[c][:, chunk t] = acc*cos + rot_half(acc)*sin."""
                    lo, hi = t * TC_W, (t + 1) * TC_W
                    blk = acc[c][:, lo:hi]
                    tmp = rtp.tile([128, TC_W], F16, name=f"rt{c}_{t}",
                                   tag="ropetmp")
                    # rotate-half copies on the scalar engine (act is idle
                    # during stage 1); muls/add all-fp16 -> DVE fast mode
                    nc.scalar.copy(tmp[0:64, :], blk[64:128, :])
                    nc.scalar.copy(tmp[64:128, :], blk[0:64, :])
                    nc.vector.tensor_mul(tmp[:], tmp[:], sint[:, lo:hi])
                    nc.vector.tensor_mul(blk, blk, cost[:, lo:hi])
                    nc.vector.tensor_add(blk, blk, tmp[:])

                for seg in range(NSEG):
                    last = seg == NSEG - 1
                    wt = {}
                    for c in C_ORDER:
                        w_tile = wp.tile([128, HB, 128], F16,
                                         name=f"w_{seg}_{c}", tag="w")
                        nc.sync.dma_start(
                            out=w_tile[:],
                            in_=w12[seg * HB * 128:(seg + 1) * HB * 128,
                                    c * 128:(c + 1) * 128]
                            .rearrange("(hb p) c -> p hb c", p=128),
                        )
                        wt[c] = w_tile
                    if seg == 1:
                        # constants are only needed from the last segment
                        # on; load them behind seg 0's weight+hid DMAs.
                        nc.sync.dma_start(out=cost[:], in_=cos2[:])
                        nc.sync.dma_start(out=sint[:], in_=sinpm[:])
                        nc.sync.dma_start(out=ident[:], in_=identd[:])
                        nc.sync.dma_start(out=ones_c[:], in_=onescd[:])
                    for t in range(TCH):
                        ht = []
                        for hb in range(HB):
                            h_tile = hp.tile([128, TC_W], F16,
                                             name=f"h_{seg}_{t}_{hb}",
                                             tag="h")
                            # gpsimd (SWDGE) queue: h loads must not sit
                            # behind next-seg w loads that are WAR-gated
                            # on this seg's compute (head-of-line block)
                            nc.gpsimd.dma_start(
                                out=h_tile[:],
                                in_=hid_t[(seg * HB + hb) * 128:
                                          (seg * HB + hb + 1) * 128,
                                          t * TC_W:(t + 1) * TC_W],
                            )
                            ht.append(h_tile)
                        for ci, c in enumerate(C_ORDER):
                            pt = ps1.tile([128, TC_W], F32,
                                          name=f"p1_{seg}_{t}_{c}", tag="ps1")
                            for hb in range(HB):
                                nc.tensor.matmul(pt[:], wt[c][:, hb, :],
                                                 ht[hb][:],
                                                 start=(hb == 0),
                                                 stop=(hb == HB - 1))
                            dst = acc_of(c)[:, t * TC_W:(t + 1) * TC_W]
                            if seg == 0:
                                nc.vector.tensor_copy(dst, pt[:])
                            else:
                                nc.vector.tensor_add(dst, dst, pt[:])
                            if last:
                                if c < Q_PER_CORE + KV_PER_CORE:
                                    rope_chunk(c, t)
                                if ci == 5:
                                    # V transposes for this chunk's 4 key
                                    # blocks; emitted a few psum groups after
                                    # the v drains so the DVE is surely ahead.
                                    for kv in range(KV_PER_CORE):
                                        for sb in range(4 * t, 4 * t + 4):
                                            ptt = ps2.tile(
                                                [128, 128], F16,
                                                name=f"pt2_{kv}_{sb}",
                                                tag="ps2")
                                            nc.tensor.transpose(
                                                ptt[:],
                                                accv[kv][:, sb * 128:
                                                         (sb + 1) * 128],
                                                ident[:],
                                            )
                                            vtile = vp.tile(
                                                [128, 128], BF16,
                                                name=f"v{kv}_{sb}",
                                                tag=f"v{kv}_{sb}")
                                            nc.scalar.copy(vtile[:], ptt[:])
                                            vnat[kv * SB + sb] = vtile

            # ---- stage 3 + 4 share the attention-output accumulator.
            # One tile per (head, chunk): keeps stage-4 LdWeights deps at
            # chunk granularity so stage 4 can start before the s3 tail.
            with tc.tile_pool(name="acco", bufs=1) as accop:
                acco = [[accop.tile([128, TC_W], BF16, name=f"acco{g}_{t}",
                                    tag=f"acco{g}_{t}") for t in range(TCH)]
                        for g in range(Q_PER_CORE)]

                # ---- stage 3: attention; batched emission, 1-batch pipeline
                with tc.tile_pool(name="probs", bufs=4) as pp, \
                     tc.tile_pool(name="recip", bufs=2) as rcp, \
                     tc.tile_pool(name="rbc", bufs=2) as rbp, \
                     tc.tile_pool(name="ps_s", bufs=2, space="PSUM") as ps_s, \
                     tc.tile_pool(name="ps_pv", bufs=2, space="PSUM") as ps_pv, \
                     tc.tile_pool(name="ps_sum", bufs=2, space="PSUM") as ps_sm:

                    # batches of 4 key blocks; PE emission per batch is
                    # [4 scores fp16] [4 pv bf16] [4 sum bf16] so psum-bank
                    # switches (and their per-group overhead) amortize.
                    # Diagonal batch LAST: its serialized gpsimd selects then
                    # hide under the (t+1) batches of pipeline cover.
                    batches = [(g, t, bb, ei)
                               for g in range(Q_PER_CORE)
                               for t in range(TCH)
                               for ei, bb in enumerate(range(t + 1))]
                    prs = {}

                    def emit_batch(bi):
                        g, t, bb, _ei = batches[bi]
                        kv = g // GROUP
                        kt = acc[Q_PER_CORE + kv]
                        prl = []
                        for p in range(2):
                            # 2-bank psum pair: two score matmuls, ONE wide
                            # exp (halves the activation-engine overhead)
                            sc = ps_s.tile([128, 2, TC_W], F32,
                                           name=f"sc_{g}_{t}_{bb}_{p}",
                                           tag="s")
                            for i in range(2):
                                sb = 4 * bb + 2 * p + i
                                nc.tensor.matmul(
                                    sc[:, i, :],
                                    kt[:, sb * 128:(sb + 1) * 128],
                                    acc[g][:, t * TC_W:(t + 1) * TC_W],
                                    start=True, stop=True,
                                )
                            pr = pp.tile([128, 2, TC_W], BF16,
                                         name=f"pr_{g}_{t}_{bb}_{p}",
                                         tag="pr")
                            # softmax scale folded into the activation
                            nc.scalar.activation(
                                pr[:], sc[:], mybir.ActivationFunctionType.Exp,
                                scale=float(SCALE))
                            for i in range(2):
                                sb = 4 * bb + 2 * p + i
                                jd = sb - 4 * t
                                if jd >= 0:
                                    # zero where key > query; gpsimd keeps
                                    # the DVE queue free for recip/final-mul
                                    nc.gpsimd.affine_select(
                                        out=pr[:, i, :], in_=pr[:, i, :],
                                        compare_op=mybir.AluOpType.is_ge,
                                        fill=0.0, base=-128 * jd,
                                        pattern=[[1, TC_W]],
                                        channel_multiplier=-1,
                                    )
                                prl.append(pr[:, i, :])
                        prs[bi] = prl

                    emit_batch(0)
                    pv = None
                    sm = None
                    pending = {}  # bi -> (g, t, pv, rcb) final-mul to emit
                    for bi, (g, t, bb, ei) in enumerate(batches):
                        if bi in pending:
                            # delayed normalization mul: broadcast has landed
                            # by now, so this never blocks the DVE queue
                            pg, pt, ppv, prcb = pending.pop(bi)
                            nc.vector.tensor_mul(acco[pg][pt][:], ppv[:],
                                                 prcb[:])
                        if bi + 1 < len(batches):
                            emit_batch(bi + 1)
                        kv = g // GROUP
                        if ei == 0:
                            pv = ps_pv.tile([128, TC_W], F32,
                                            name=f"pv_{g}_{t}", tag="pv")
                            sm = ps_sm.tile([1, TC_W], F32,
                                            name=f"sm_{g}_{t}", tag="sum")
                        prl = prs.pop(bi)
                        for i in range(4):
                            sb = 4 * bb + i
                            first = ei == 0 and i == 0
                            lastm = ei == t and i == 3
                            nc.tensor.matmul(pv[:], vnat[kv * SB + sb][:],
                                             prl[i], start=first,
                                             stop=lastm)
                        for i in range(4):
                            sb = 4 * bb + i
                            first = ei == 0 and i == 0
                            lastm = ei == t and i == 3
                            nc.tensor.matmul(sm[:], ones_c[:], prl[i],
                                             start=first,
                                             stop=lastm)
                        if ei == t:
                            # normalization chain, fully off the PE
                            rc = rcp.tile([1, TC_W], F32,
                                          name=f"rc_{g}_{t}", tag="rc")
                            nc.vector.reciprocal_approx_fast(rc[:], sm[:])
                            rcb = rbp.tile([128, TC_W], F32,
                                           name=f"rcb_{g}_{t}", tag="rcb")
                            nc.gpsimd.partition_broadcast(rcb[:], rc[:])
                            pending[bi + 2] = (g, t, pv, rcb)
                    for pg, pt, ppv, prcb in pending.values():
                        nc.vector.tensor_mul(acco[pg][pt][:], ppv[:], prcb[:])

                # ---- stage 4: out[t, n] = sum_g attn_g^T @ wo_g  (bf16)
                with tc.tile_pool(name="wop", bufs=8) as wops, \
                     tc.tile_pool(name="outp", bufs=4) as op, \
                     tc.tile_pool(name="ps4", bufs=4, space="PSUM") as ps4:
                    for n in range(H // TC_W):
                        wq4 = []
                        for q4 in range(4):
                            wn = wops.tile([128, 2, TC_W], BF16,
                                           name=f"wo_{n}_{q4}", tag="wo")
                            nc.sync.dma_start(
                                out=wn[:],
                                in_=wo[q4 * 256:(q4 + 1) * 256,
                                       n * TC_W:(n + 1) * TC_W]
                                .rearrange("(g p) c -> p g c", p=128),
                            )
                            wq4.append(wn)
                        for tb in range(SB):
                            po = ps4.tile([128, TC_W], F32,
                                          name=f"po_{n}_{tb}", tag="po")
                            for g in range(Q_PER_CORE):
                                nc.tensor.matmul(
                                    po[:],
                                    acco[g][tb // 4][:, (tb % 4) * 128:
                                                     (tb % 4 + 1) * 128],
                                    wq4[g // 2][:, g % 2, :],
                                    start=(g == 0),
                                    stop=(g == Q_PER_CORE - 1),
                                )
                            ot = op.tile([128, TC_W], F32,
                                         name=f"ot_{n}_{tb}", tag="ot")
                            nc.scalar.copy(ot[:], po[:])
                            nc.gpsimd.dma_start(
                                out=out[tb * 128:(tb + 1) * 128,
                                        n * TC_W:(n + 1) * TC_W],
                                in_=ot[:],
                            )

    nc.compile()
    return nc


def _get_compiled():
    global _compiled
    if _compiled is None:
        _compiled = _build()
    return _compiled


_EVEN_ODD = np.concatenate([np.arange(0, HD, 2), np.arange(1, HD, 2)])


def _prep_core_inputs(hidden_states, positions, wqkv, wo):
    """Returns list of 8 in_maps (core c = 4*b + t)."""
    inv_freq = ROPE_BASE ** (-np.arange(0, HD, 2, dtype=np.float32) / HD)
    ident = np.eye(128, dtype=np.float16)
    ones_c = np.ones((128, 1), dtype=ml_dtypes.bfloat16)

    per_batch = []
    for b in range(B):
        hid_t = np.ascontiguousarray(hidden_states[b].T.astype(np.float16))
        ang = positions[b].astype(np.float32)[:, None] * inv_freq[None, :]
        cos = np.cos(ang).T.astype(np.float32)  # [64, S]
        sin = np.sin(ang).T.astype(np.float32)
        cos2 = np.ascontiguousarray(
            np.concatenate([cos, cos], axis=0).astype(np.float16))
        sinpm = np.ascontiguousarray(
            np.concatenate([-sin, sin], axis=0).astype(np.float16))
        per_batch.append((hid_t, cos2, sinpm))

    in_maps = []
    for c in range(N_CORES):
        b, t = c // TP, c % TP
        hid_t, cos2, sinpm = per_batch[b]
        blocks = []
        for gh in range(Q_PER_CORE):  # q heads, permuted (softmax scale is
            h = Q_PER_CORE * t + gh   # applied in the exp activation)
            blocks.append(wqkv[:, h * HD:(h + 1) * HD][:, _EVEN_ODD])
        for m in range(KV_PER_CORE):  # k heads, permuted
            h = KV_PER_CORE * t + m
            blocks.append(
                wqkv[:, NH * HD + h * HD: NH * HD + (h + 1) * HD][:, _EVEN_ODD])
        for m in range(KV_PER_CORE):  # v heads, natural
            h = KV_PER_CORE * t + m
            base = (NH + NKV) * HD
            blocks.append(wqkv[:, base + h * HD: base + (h + 1) * HD])
        w12 = np.ascontiguousarray(
            np.concatenate(blocks, axis=1).astype(np.float16))
        wo_shard = np.ascontiguousarray(
            wo[Q_PER_CORE * HD * t: Q_PER_CORE * HD * (t + 1), :]
            .astype(ml_dtypes.bfloat16))
        in_maps.append({
            "hid_t": hid_t, "w12": w12, "wo": wo_shard,
            "cos2": cos2, "sinpm": sinpm,
            "identd": ident, "onescd": ones_c,
        })
    return in_maps


def kernel(hidden_states, positions, wqkv, wo):
    hidden_states = np.asarray(hidden_states)
    positions = np.asarray(positions)
    wqkv = np.asarray(wqkv)
    wo = np.asarray(wo)
    nc = _get_compiled()
    in_maps = _prep_core_inputs(hidden_states, positions, wqkv, wo)
    res = run_bass_kernel_spmd(nc, in_maps, list(range(N_CORES)))
    full = np.zeros((B, S, H), dtype=np.float32)
    for c in range(N_CORES):
        full[c // TP] += res.results[c]["out"]
    return full


# revision 28
# speedup vs baseline: 1.1948x; 1.1943x over previous
"""Mixtral GQA attention (B=2, S=2048, H=4096, 32 q heads / 8 kv heads,
interleaved RoPE, causal; sliding window 4096 >= S so it is plain causal)
on 8 Trainium2 NeuronCores.

Sharding: DP=2 over batch x TP=4 over kv-head pairs. Core c = 4*b + t
handles batch b, kv heads {2t, 2t+1}, q heads [8t, 8t+8). Each core
computes qkv projection (transposed layout), RoPE, attention, and its
partial of the wo projection; the host sums the 4 partials per batch.

Perf notes (v4):
 - Everything computed transposed ([feature, token]); contraction on
   partitions. The q/k path runs in fp16 (scores err ~1e-3: fine), with
   the softmax 1/sqrt(d) folded into the exp activation's scale so the
   unscaled q weights stay clear of fp16 subnormals. probs / V / attn
   out / wo are bf16 (exp needs bf16 range). fp32 PSUM accumulation
   everywhere. fp16 halves LdWeights time, SBUF footprint, and DMA.
 - Stage 1 uses 2 H-segments -> 16-matmul PSUM accumulation groups
   (per-group overheads amortize; measured bank-switch cost ~95ns).
 - RoPE and V transposes interleave chunk-wise into the last H-segment
   (k/v feature blocks computed first), so the PE flows straight from
   projection into attention. RoPE runs on fp16 tiles at the DVE's
   2-byte fast mode; rotate-half copies go to the scalar engine.
 - Attention PE emission per 4-key-block batch is [4 scores][4 pv]
   [4 sum] (psum-bank switches amortize), software-pipelined one batch
   ahead across (head, chunk) boundaries to hide exp/mask latency.
 - Causal masking via gpsimd affine_select (keeps the DVE queue free:
   its head-of-line blocking caused stalls at attention entry/exit).
 - Softmax denominators: ones-column matmul; reciprocal_approx_fast;
   gpsimd partition_broadcast; final normalization mul on DVE delayed
   two batches so it never blocks ahead of mask work. attn output is
   one tile per (head, chunk) so stage 4's deps are chunk-granular.
 - h-tile DMAs ride the gpsimd SWDGE queue: on the sync queue they sat
   behind WAR-gated next-segment weight loads (head-of-line blocking).
"""

import sys

sys.path.insert(0, "/opt/trn_rl_repo")

import numpy as np
import ml_dtypes

import concourse.bass as bass  # noqa: F401
import concourse.mybir as mybir
import concourse.tile as tile
from concourse import bacc
from concourse.bass_utils import run_bass_kernel_spmd

F32 = mybir.dt.float32
F32R = mybir.dt.float32r
BF16 = mybir.dt.bfloat16
F16 = mybir.dt.float16

B = 2
S = 2048
H = 4096
NH = 32
NKV = 8
HD = 128
GROUP = NH // NKV
ROPE_BASE = 10000.0
SCALE = HD**-0.5

N_CORES = 8
TP = 4  # kv-head-pair groups
Q_PER_CORE = 8
KV_PER_CORE = 2

NC_BLK = Q_PER_CORE + 2 * KV_PER_CORE  # 12 feature blocks of 128 in stage 1
NSEG = 2  # contraction (H) segments
HB = H // 128 // NSEG  # h-blocks per segment = 16
TCH = 4  # token chunks
TC_W = S // TCH  # 512
SB = S // 128  # 16 key blocks

# stage-1 c-block order: k heads, v heads, then q heads (so rope-k and
# V transposes start as early as possible inside the last segment)
C_ORDER = [Q_PER_CORE, Q_PER_CORE + 1, Q_PER_CORE + 2, Q_PER_CORE + 3] + list(
    range(Q_PER_CORE)
)

_compiled = None


def _build():
    nc = bacc.Bacc("TRN2", target_bir_lowering=False, debug=False,
                   num_devices=N_CORES)

    hid_t = nc.declare_dram_parameter("hid_t", [H, S], F16, isOutput=False)
    w12 = nc.declare_dram_parameter("w12", [H, NC_BLK * 128], F16, isOutput=False)
    wo = nc.declare_dram_parameter("wo", [Q_PER_CORE * 128, H], BF16,
                                   isOutput=False)
    cos2 = nc.declare_dram_parameter("cos2", [128, S], F16, isOutput=False)
    sinpm = nc.declare_dram_parameter("sinpm", [128, S], F16, isOutput=False)
    identd = nc.declare_dram_parameter("identd", [128, 128], F16, isOutput=False)
    onescd = nc.declare_dram_parameter("onescd", [128, 1], BF16, isOutput=False)
    out = nc.declare_dram_parameter("out", [S, H], F32, isOutput=True)

    with tile.TileContext(nc) as tc:
        with tc.tile_pool(name="consts", bufs=1) as consts, \
             tc.tile_pool(name="acc", bufs=1) as accp:
            ident = consts.tile([128, 128], F16, name="ident", tag="ident")
            ones_c = consts.tile([128, 1], BF16, name="ones_c", tag="ones_c")
            cost = consts.tile([128, S], F16, name="cost", tag="cost")
            sint = consts.tile([128, S], F16, name="sint", tag="sint")
            warm = consts.tile([128, 1], F16, name="warm", tag="warm")

            # warm the Exp activation table long before attention needs it
            nc.sync.dma_start(out=warm[:], in_=identd[:, 0:1])
            nc.scalar.activation(warm[:], warm[:],
                                 mybir.ActivationFunctionType.Exp)

            # q/k accumulators in fp16 (scores precision ~1e-3, plenty)
            acc = [accp.tile([128, S], F16, name=f"acc{c}", tag=f"acc{c}")
                   for c in range(Q_PER_CORE + KV_PER_CORE)]

            vnat = [None] * (KV_PER_CORE * SB)

            # ---- stage 1: qkv^T = w12^T @ hid_t over 2 H-segments.
            # In the last segment, rope + V transposes interleave chunk-wise.
            with tc.tile_pool(name="accv", bufs=1) as accvp, \
                 tc.tile_pool(name="wseg", bufs=16) as wp, \
                 tc.tile_pool(name="hidt", bufs=32) as hp, \
                 tc.tile_pool(name="ropet", bufs=3) as rtp, \
                 tc.tile_pool(name="vnatp", bufs=1) as vp, \
                 tc.tile_pool(name="ps1", bufs=3, space="PSUM") as ps1, \
                 tc.tile_pool(name="ps2", bufs=2, space="PSUM") as ps2:

                accv = [accvp.tile([128, S], F16, name=f"accv{m}",
                                   tag=f"accv{m}") for m in range(KV_PER_CORE)]

                def acc_of(c):
                    if c < Q_PER_CORE + KV_PER_CORE:
                        return acc[c]
                    return accv[c - Q_PER_CORE - KV_PER_CORE]

                def rope_chunk(c, t):
                    """acc[c][:, chunk t] = acc*cos + rot_half(acc)*sin."""
                    lo, hi = t * TC_W, (t + 1) * TC_W
                    blk = acc[c][:, lo:hi]
                    tmp = rtp.tile([128, TC_W], F16, name=f"rt{c}_{t}",
                                   tag="ropetmp")
                    # rotate-half copies on the scalar engine (act is idle
                    # during stage 1); muls/add all-fp16 -> DVE fast mode
                    nc.scalar.copy(tmp[0:64, :], blk[64:128, :])
                    nc.scalar.copy(tmp[64:128, :], blk[0:64, :])
                    nc.vector.tensor_mul(tmp[:], tmp[:], sint[:, lo:hi])
                    nc.vector.tensor_mul(blk, blk, cost[:, lo:hi])
                    nc.vector.tensor_add(blk, blk, tmp[:])

                for seg in range(NSEG):
                    last = seg == NSEG - 1
                    wt = {}
                    for c in C_ORDER:
                        w_tile = wp.tile([128, HB, 128], F16,
                                         name=f"w_{seg}_{c}", tag="w")
                        nc.sync.dma_start(
                            out=w_tile[:],
                            in_=w12[seg * HB * 128:(seg + 1) * HB * 128,
                                    c * 128:(c + 1) * 128]
                            .rearrange("(hb p) c -> p hb c", p=128),
                        )
                        wt[c] = w_tile
                        if seg == 0 and c == C_ORDER[0]:
                            # even h tiles of the first chunk ride the sync
                            # queue right behind the first w tile; odd ones
                            # go through SWDGE as usual. Splitting across
                            # both queues halves the serial descriptor time
                            # gating the very first psum group.
                            ht00 = {}
                            for hb in range(0, HB, 2):
                                h_tile = hp.tile([128, TC_W], F16,
                                                 name=f"h_0_0_{hb}", tag="h")
                                nc.sync.dma_start(
                                    out=h_tile[:],
                                    in_=hid_t[hb * 128:(hb + 1) * 128,
                                              0:TC_W],
                                )
                                ht00[hb] = h_tile
                    if seg == 1:
                        # constants are only needed from the last segment
                        # on; load them behind seg 0's weight+hid DMAs.
                        nc.sync.dma_start(out=cost[:], in_=cos2[:])
                        nc.sync.dma_start(out=sint[:], in_=sinpm[:])
                        nc.sync.dma_start(out=ident[:], in_=identd[:])
                        nc.sync.dma_start(out=ones_c[:], in_=onescd[:])
                    for t in range(TCH):
                        ht = []
                        for hb in range(HB):
                            if seg == 0 and t == 0 and hb % 2 == 0:
                                ht.append(ht00[hb])
                                continue
                            h_tile = hp.tile([128, TC_W], F16,
                                             name=f"h_{seg}_{t}_{hb}",
                                             tag="h")
                            # gpsimd (SWDGE) queue: h loads must not sit
                            # behind next-seg w loads that are WAR-gated
                            # on this seg's compute (head-of-line block)
                            nc.gpsimd.dma_start(
                                out=h_tile[:],
                                in_=hid_t[(seg * HB + hb) * 128:
                                          (seg * HB + hb + 1) * 128,
                                          t * TC_W:(t + 1) * TC_W],
                            )
                            ht.append(h_tile)
                        for ci, c in enumerate(C_ORDER):
                            pt = ps1.tile([128, TC_W], F32,
                                          name=f"p1_{seg}_{t}_{c}", tag="ps1")
                            for hb in range(HB):
                                nc.tensor.matmul(pt[:], wt[c][:, hb, :],
                                                 ht[hb][:],
                                                 start=(hb == 0),
                                                 stop=(hb == HB - 1))
                            dst = acc_of(c)[:, t * TC_W:(t + 1) * TC_W]
                            if seg == 0:
                                nc.vector.tensor_copy(dst, pt[:])
                            else:
                                nc.vector.tensor_add(dst, dst, pt[:])
                            if last:
                                if c < Q_PER_CORE + KV_PER_CORE:
                                    rope_chunk(c, t)
                                if ci == 5:
                                    # V transposes for this chunk's 4 key
                                    # blocks; emitted a few psum groups after
                                    # the v drains so the DVE is surely ahead.
                                    for kv in range(KV_PER_CORE):
                                        for sb in range(4 * t, 4 * t + 4):
                                            ptt = ps2.tile(
                                                [128, 128], F16,
                                                name=f"pt2_{kv}_{sb}",
                                                tag="ps2")
                                            nc.tensor.transpose(
                                                ptt[:],
                                                accv[kv][:, sb * 128:
                                                         (sb + 1) * 128],
                                                ident[:],
                                            )
                                            vtile = vp.tile(
                                                [128, 128], BF16,
                                                name=f"v{kv}_{sb}",
                                                tag=f"v{kv}_{sb}")
                                            nc.scalar.copy(vtile[:], ptt[:])
                                            vnat[kv * SB + sb] = vtile

            # ---- stage 3 + 4 share the attention-output accumulator.
            # One tile per (head, chunk): keeps stage-4 LdWeights deps at
            # chunk granularity so stage 4 can start before the s3 tail.
            with tc.tile_pool(name="acco", bufs=1) as accop:
                acco = [[accop.tile([128, TC_W], BF16, name=f"acco{g}_{t}",
                                    tag=f"acco{g}_{t}") for t in range(TCH)]
                        for g in range(Q_PER_CORE)]

                # ---- stage 3: attention; batched emission, 1-batch pipeline
                with tc.tile_pool(name="probs", bufs=4) as pp, \
                     tc.tile_pool(name="recip", bufs=2) as rcp, \
                     tc.tile_pool(name="rbc", bufs=2) as rbp, \
                     tc.tile_pool(name="ps_s", bufs=2, space="PSUM") as ps_s, \
                     tc.tile_pool(name="ps_pv", bufs=2, space="PSUM") as ps_pv, \
                     tc.tile_pool(name="ps_sum", bufs=2, space="PSUM") as ps_sm:

                    # batches of 4 key blocks; PE emission per batch is
                    # [4 scores fp16] [4 pv bf16] [4 sum bf16] so psum-bank
                    # switches (and their per-group overhead) amortize.
                    # Diagonal batch LAST: its serialized gpsimd selects then
                    # hide under the (t+1) batches of pipeline cover. For the
                    # very last (head, chunk) there is no cover after it, so
                    # that one runs its diagonal batch FIRST instead.
                    def bb_order(g, t):
                        if (g, t) == (Q_PER_CORE - 1, TCH - 1):
                            return [t] + list(range(t))
                        return list(range(t + 1))
                    batches = [(g, t, bb, ei)
                               for g in range(Q_PER_CORE)
                               for t in range(TCH)
                               for ei, bb in enumerate(bb_order(g, t))]
                    prs = {}

                    def emit_batch(bi):
                        g, t, bb, _ei = batches[bi]
                        kv = g // GROUP
                        kt = acc[Q_PER_CORE + kv]
                        prl = []
                        for p in range(2):
                            # 2-bank psum pair: two score matmuls, ONE wide
                            # exp (halves the activation-engine overhead)
                            sc = ps_s.tile([128, 2, TC_W], F32,
                                           name=f"sc_{g}_{t}_{bb}_{p}",
                                           tag="s")
                            for i in range(2):
                                sb = 4 * bb + 2 * p + i
                                nc.tensor.matmul(
                                    sc[:, i, :],
                                    kt[:, sb * 128:(sb + 1) * 128],
                                    acc[g][:, t * TC_W:(t + 1) * TC_W],
                                    start=True, stop=True,
                                )
                            pr = pp.tile([128, 2, TC_W], BF16,
                                         name=f"pr_{g}_{t}_{bb}_{p}",
                                         tag="pr")
                            # softmax scale folded into the activation
                            nc.scalar.activation(
                                pr[:], sc[:], mybir.ActivationFunctionType.Exp,
                                scale=float(SCALE))
                            for i in range(2):
                                sb = 4 * bb + 2 * p + i
                                jd = sb - 4 * t
                                if jd >= 0:
                                    # zero where key > query; gpsimd keeps
                                    # the DVE queue free for recip/final-mul
                                    nc.gpsimd.affine_select(
                                        out=pr[:, i, :], in_=pr[:, i, :],
                                        compare_op=mybir.AluOpType.is_ge,
                                        fill=0.0, base=-128 * jd,
                                        pattern=[[1, TC_W]],
                                        channel_multiplier=-1,
                                    )
                                prl.append(pr[:, i, :])
                        prs[bi] = prl

                    emit_batch(0)
                    pv = None
                    sm = None
                    pending = {}  # bi -> (g, t, pv, rcb) final-mul to emit
                    for bi, (g, t, bb, ei) in enumerate(batches):
                        if bi in pending:
                            # delayed normalization mul: broadcast has landed
                            # by now, so this never blocks the DVE queue
                            pg, pt, ppv, prcb = pending.pop(bi)
                            nc.vector.tensor_mul(acco[pg][pt][:], ppv[:],
                                                 prcb[:])
                        if bi + 1 < len(batches):
                            emit_batch(bi + 1)
                        kv = g // GROUP
                        if ei == 0:
                            pv = ps_pv.tile([128, TC_W], F32,
                                            name=f"pv_{g}_{t}", tag="pv")
                            sm = ps_sm.tile([1, TC_W], F32,
                                            name=f"sm_{g}_{t}", tag="sum")
                        prl = prs.pop(bi)
                        for i in range(4):
                            sb = 4 * bb + i
                            first = ei == 0 and i == 0
                            lastm = ei == t and i == 3
                            nc.tensor.matmul(pv[:], vnat[kv * SB + sb][:],
                                             prl[i], start=first,
                                             stop=lastm)
                        for i in range(4):
                            sb = 4 * bb + i
                            first = ei == 0 and i == 0
                            lastm = ei == t and i == 3
                            nc.tensor.matmul(sm[:], ones_c[:], prl[i],
                                             start=first,
                                             stop=lastm)
                        if ei == t:
                            # normalization chain, fully off the PE
                            rc = rcp.tile([1, TC_W], F32,
                                          name=f"rc_{g}_{t}", tag="rc")
                            nc.vector.reciprocal_approx_fast(rc[:], sm[:])
                            rcb = rbp.tile([128, TC_W], F32,
                                           name=f"rcb_{g}_{t}", tag="rcb")
                            nc.gpsimd.partition_broadcast(rcb[:], rc[:])
                            pending[bi + 2] = (g, t, pv, rcb)
                    for pg, pt, ppv, prcb in pending.values():
                        nc.vector.tensor_mul(acco[pg][pt][:], ppv[:], prcb[:])

                # ---- stage 4: out[t, n] = sum_g attn_g^T @ wo_g  (bf16)
                with tc.tile_pool(name="wop", bufs=8) as wops, \
                     tc.tile_pool(name="outp", bufs=4) as op, \
                     tc.tile_pool(name="ps4", bufs=4, space="PSUM") as ps4:
                    for n in range(H // TC_W):
                        wq4 = []
                        for q4 in range(4):
                            wn = wops.tile([128, 2, TC_W], BF16,
                                           name=f"wo_{n}_{q4}", tag="wo")
                            nc.sync.dma_start(
                                out=wn[:],
                                in_=wo[q4 * 256:(q4 + 1) * 256,
                                       n * TC_W:(n + 1) * TC_W]
                                .rearrange("(g p) c -> p g c", p=128),
                            )
                            wq4.append(wn)
                        for tb in range(SB):
                            po = ps4.tile([128, TC_W], F32,
                                          name=f"po_{n}_{tb}", tag="po")
                            for g in range(Q_PER_CORE):
                                nc.tensor.matmul(
                                    po[:],
                                    acco[g][tb // 4][:, (tb % 4) * 128:
                                                     (tb % 4 + 1) * 128],
                                    wq4[g // 2][:, g % 2, :],
                                    start=(g == 0),
                                    stop=(g == Q_PER_CORE - 1),
                                )
                            ot = op.tile([128, TC_W], F32,
                                         name=f"ot_{n}_{tb}", tag="ot")
                            nc.scalar.copy(ot[:], po[:])
                            nc.gpsimd.dma_start(
                                out=out[tb * 128:(tb + 1) * 128,
                                        n * TC_W:(n + 1) * TC_W],
                                in_=ot[:],
                            )

    nc.compile()
    return nc


def _get_compiled():
    global _compiled
    if _compiled is None:
        _compiled = _build()
    return _compiled


_EVEN_ODD = np.concatenate([np.arange(0, HD, 2), np.arange(1, HD, 2)])


def _prep_core_inputs(hidden_states, positions, wqkv, wo):
    """Returns list of 8 in_maps (core c = 4*b + t)."""
    inv_freq = ROPE_BASE ** (-np.arange(0, HD, 2, dtype=np.float32) / HD)
    ident = np.eye(128, dtype=np.float16)
    ones_c = np.ones((128, 1), dtype=ml_dtypes.bfloat16)

    per_batch = []
    for b in range(B):
        hid_t = np.ascontiguousarray(hidden_states[b].T.astype(np.float16))
        ang = positions[b].astype(np.float32)[:, None] * inv_freq[None, :]
        cos = np.cos(ang).T.astype(np.float32)  # [64, S]
        sin = np.sin(ang).T.astype(np.float32)
        cos2 = np.ascontiguousarray(
            np.concatenate([cos, cos], axis=0).astype(np.float16))
        sinpm = np.ascontiguousarray(
            np.concatenate([-sin, sin], axis=0).astype(np.float16))
        per_batch.append((hid_t, cos2, sinpm))

    in_maps = []
    for c in range(N_CORES):
        b, t = c // TP, c % TP
        hid_t, cos2, sinpm = per_batch[b]
        blocks = []
        for gh in range(Q_PER_CORE):  # q heads, permuted (softmax scale is
            h = Q_PER_CORE * t + gh   # applied in the exp activation)
            blocks.append(wqkv[:, h * HD:(h + 1) * HD][:, _EVEN_ODD])
        for m in range(KV_PER_CORE):  # k heads, permuted
            h = KV_PER_CORE * t + m
            blocks.append(
                wqkv[:, NH * HD + h * HD: NH * HD + (h + 1) * HD][:, _EVEN_ODD])
        for m in range(KV_PER_CORE):  # v heads, natural
            h = KV_PER_CORE * t + m
            base = (NH + NKV) * HD
            blocks.append(wqkv[:, base + h * HD: base + (h + 1) * HD])
        w12 = np.ascontiguousarray(
            np.concatenate(blocks, axis=1).astype(np.float16))
        wo_shard = np.ascontiguousarray(
            wo[Q_PER_CORE * HD * t: Q_PER_CORE * HD * (t + 1), :]
            .astype(ml_dtypes.bfloat16))
        in_maps.append({
            "hid_t": hid_t, "w12": w12, "wo": wo_shard,
            "cos2": cos2, "sinpm": sinpm,
            "identd": ident, "onescd": ones_c,
        })
    return in_maps


def kernel(hidden_states, positions, wqkv, wo):
    hidden_states = np.asarray(hidden_states)
    positions = np.asarray(positions)
    wqkv = np.asarray(wqkv)
    wo = np.asarray(wo)
    nc = _get_compiled()
    in_maps = _prep_core_inputs(hidden_states, positions, wqkv, wo)
    res = run_bass_kernel_spmd(nc, in_maps, list(range(N_CORES)))
    full = np.zeros((B, S, H), dtype=np.float32)
    for c in range(N_CORES):
        full[c // TP] += res.results[c]["out"]
    return full


# revision 32
# speedup vs baseline: 1.2094x; 1.0122x over previous
"""Mixtral GQA attention (B=2, S=2048, H=4096, 32 q heads / 8 kv heads,
interleaved RoPE, causal; sliding window 4096 >= S so it is plain causal)
on 8 Trainium2 NeuronCores.

Sharding: DP=2 over batch x TP=4 over kv-head pairs. Core c = 4*b + t
handles batch b, kv heads {2t, 2t+1}, q heads [8t, 8t+8). Each core
computes qkv projection (transposed layout), RoPE, attention, and its
partial of the wo projection; the host sums the 4 partials per batch.

Perf notes (v4):
 - Everything computed transposed ([feature, token]); contraction on
   partitions. The q/k path runs in fp16 (scores err ~1e-3: fine), with
   the softmax 1/sqrt(d) folded into the exp activation's scale so the
   unscaled q weights stay clear of fp16 subnormals. probs / V / attn
   out / wo are bf16 (exp needs bf16 range). fp32 PSUM accumulation
   everywhere. fp16 halves LdWeights time, SBUF footprint, and DMA.
 - Stage 1 uses 2 H-segments -> 16-matmul PSUM accumulation groups
   (per-group overheads amortize; measured bank-switch cost ~95ns).
 - RoPE and V transposes interleave chunk-wise into the last H-segment
   (k/v feature blocks computed first), so the PE flows straight from
   projection into attention. RoPE runs on fp16 tiles at the DVE's
   2-byte fast mode; rotate-half copies go to the scalar engine.
 - Attention PE emission per 4-key-block batch is [4 scores][4 pv]
   [4 sum] (psum-bank switches amortize), software-pipelined one batch
   ahead across (head, chunk) boundaries to hide exp/mask latency.
 - Causal masking via gpsimd affine_select (keeps the DVE queue free:
   its head-of-line blocking caused stalls at attention entry/exit).
 - Softmax denominators: ones-column matmul; reciprocal_approx_fast;
   gpsimd partition_broadcast; final normalization mul on DVE delayed
   two batches so it never blocks ahead of mask work. attn output is
   one tile per (head, chunk) so stage 4's deps are chunk-granular.
 - h-tile DMAs ride the gpsimd SWDGE queue: on the sync queue they sat
   behind WAR-gated next-segment weight loads (head-of-line blocking).
"""

import sys

sys.path.insert(0, "/opt/trn_rl_repo")

import numpy as np
import ml_dtypes

import concourse.bass as bass  # noqa: F401
import concourse.mybir as mybir
import concourse.tile as tile
from concourse import bacc
from concourse.bass_utils import run_bass_kernel_spmd

F32 = mybir.dt.float32
F32R = mybir.dt.float32r
BF16 = mybir.dt.bfloat16
F16 = mybir.dt.float16

B = 2
S = 2048
H = 4096
NH = 32
NKV = 8
HD = 128
GROUP = NH // NKV
ROPE_BASE = 10000.0
SCALE = HD**-0.5

N_CORES = 8
TP = 4  # kv-head-pair groups
Q_PER_CORE = 8
KV_PER_CORE = 2

NC_BLK = Q_PER_CORE + 2 * KV_PER_CORE  # 12 feature blocks of 128 in stage 1
NSEG = 2  # contraction (H) segments
HB = H // 128 // NSEG  # h-blocks per segment = 16
TCH = 4  # token chunks
TC_W = S // TCH  # 512
SB = S // 128  # 16 key blocks

# stage-1 c-block order: k heads, v heads, then q heads (so rope-k and
# V transposes start as early as possible inside the last segment)
C_ORDER = [Q_PER_CORE, Q_PER_CORE + 1, Q_PER_CORE + 2, Q_PER_CORE + 3] + list(
    range(Q_PER_CORE)
)

_compiled = None


def _build():
    nc = bacc.Bacc("TRN2", target_bir_lowering=False, debug=False,
                   num_devices=N_CORES)

    hid_t = nc.declare_dram_parameter("hid_t", [H, S], F16, isOutput=False)
    w12 = nc.declare_dram_parameter("w12", [H, NC_BLK * 128], F16, isOutput=False)
    wo = nc.declare_dram_parameter("wo", [Q_PER_CORE * 128, H], BF16,
                                   isOutput=False)
    cos2 = nc.declare_dram_parameter("cos2", [128, S], F16, isOutput=False)
    sinpm = nc.declare_dram_parameter("sinpm", [128, S], F16, isOutput=False)
    identd = nc.declare_dram_parameter("identd", [128, 128], F16, isOutput=False)
    onescd = nc.declare_dram_parameter("onescd", [128, 1], BF16, isOutput=False)
    out = nc.declare_dram_parameter("out", [S, H], F32, isOutput=True)

    with tile.TileContext(nc) as tc:
        with tc.tile_pool(name="consts", bufs=1) as consts, \
             tc.tile_pool(name="acc", bufs=1) as accp:
            ident = consts.tile([128, 128], F16, name="ident", tag="ident")
            ones_c = consts.tile([128, 1], BF16, name="ones_c", tag="ones_c")
            cost = consts.tile([128, S], F16, name="cost", tag="cost")
            sint = consts.tile([128, S], F16, name="sint", tag="sint")
            warm = consts.tile([128, 1], F16, name="warm", tag="warm")

            # warm the Exp activation table long before attention needs it
            nc.sync.dma_start(out=warm[:], in_=identd[:, 0:1])
            nc.scalar.activation(warm[:], warm[:],
                                 mybir.ActivationFunctionType.Exp)

            # q/k accumulators in fp16 (scores precision ~1e-3, plenty)
            acc = [accp.tile([128, S], F16, name=f"acc{c}", tag=f"acc{c}")
                   for c in range(Q_PER_CORE + KV_PER_CORE)]

            vnat = [None] * (KV_PER_CORE * SB)

            # ---- stage 1: qkv^T = w12^T @ hid_t over 2 H-segments.
            # In the last segment, rope + V transposes interleave chunk-wise.
            with tc.tile_pool(name="accv", bufs=1) as accvp, \
                 tc.tile_pool(name="wseg", bufs=16) as wp, \
                 tc.tile_pool(name="hidt", bufs=32) as hp, \
                 tc.tile_pool(name="ropet", bufs=3) as rtp, \
                 tc.tile_pool(name="vnatp", bufs=1) as vp, \
                 tc.tile_pool(name="ps1", bufs=3, space="PSUM") as ps1, \
                 tc.tile_pool(name="ps2", bufs=2, space="PSUM") as ps2:

                accv = [accvp.tile([128, S], F16, name=f"accv{m}",
                                   tag=f"accv{m}") for m in range(KV_PER_CORE)]

                def acc_of(c):
                    if c < Q_PER_CORE + KV_PER_CORE:
                        return acc[c]
                    return accv[c - Q_PER_CORE - KV_PER_CORE]

                def rope_chunk(c, t):
                    """acc[c][:, chunk t] = acc*cos + rot_half(acc)*sin."""
                    lo, hi = t * TC_W, (t + 1) * TC_W
                    blk = acc[c][:, lo:hi]
                    tmp = rtp.tile([128, TC_W], F16, name=f"rt{c}_{t}",
                                   tag="ropetmp")
                    # rotate-half copies on the scalar engine (act is idle
                    # during stage 1); muls/add all-fp16 -> DVE fast mode.
                    # Last chunk's copies go to DVE instead: the act queue
                    # must be empty when attention's first exps arrive.
                    cp = nc.vector.tensor_copy if t == TCH - 1 \
                        else nc.scalar.copy
                    cp(tmp[0:64, :], blk[64:128, :])
                    cp(tmp[64:128, :], blk[0:64, :])
                    nc.vector.tensor_mul(tmp[:], tmp[:], sint[:, lo:hi])
                    nc.vector.tensor_mul(blk, blk, cost[:, lo:hi])
                    nc.vector.tensor_add(blk, blk, tmp[:])

                for seg in range(NSEG):
                    last = seg == NSEG - 1
                    wt = {}
                    for c in C_ORDER:
                        w_tile = wp.tile([128, HB, 128], F16,
                                         name=f"w_{seg}_{c}", tag="w")
                        nc.sync.dma_start(
                            out=w_tile[:],
                            in_=w12[seg * HB * 128:(seg + 1) * HB * 128,
                                    c * 128:(c + 1) * 128]
                            .rearrange("(hb p) c -> p hb c", p=128),
                        )
                        wt[c] = w_tile
                        if seg == 0 and c == C_ORDER[0]:
                            # even h tiles of the first chunk ride the sync
                            # queue right behind the first w tile; odd ones
                            # go through SWDGE as usual. Splitting across
                            # both queues halves the serial descriptor time
                            # gating the very first psum group.
                            ht00 = {}
                            for hb in range(0, HB, 2):
                                h_tile = hp.tile([128, TC_W], F16,
                                                 name=f"h_0_0_{hb}", tag="h")
                                nc.sync.dma_start(
                                    out=h_tile[:],
                                    in_=hid_t[hb * 128:(hb + 1) * 128,
                                              0:TC_W],
                                )
                                ht00[hb] = h_tile
                    if seg == 1:
                        # constants are only needed from the last segment
                        # on; load them behind seg 0's weight+hid DMAs.
                        nc.sync.dma_start(out=cost[:], in_=cos2[:])
                        nc.sync.dma_start(out=sint[:], in_=sinpm[:])
                        nc.sync.dma_start(out=ident[:], in_=identd[:])
                        nc.sync.dma_start(out=ones_c[:], in_=onescd[:])
                    for t in range(TCH):
                        ht = []
                        for hb in range(HB):
                            if seg == 0 and t == 0 and hb % 2 == 0:
                                ht.append(ht00[hb])
                                continue
                            h_tile = hp.tile([128, TC_W], F16,
                                             name=f"h_{seg}_{t}_{hb}",
                                             tag="h")
                            # gpsimd (SWDGE) queue: h loads must not sit
                            # behind next-seg w loads that are WAR-gated
                            # on this seg's compute (head-of-line block)
                            nc.gpsimd.dma_start(
                                out=h_tile[:],
                                in_=hid_t[(seg * HB + hb) * 128:
                                          (seg * HB + hb + 1) * 128,
                                          t * TC_W:(t + 1) * TC_W],
                            )
                            ht.append(h_tile)
                        for ci, c in enumerate(C_ORDER):
                            pt = ps1.tile([128, TC_W], F32,
                                          name=f"p1_{seg}_{t}_{c}", tag="ps1")
                            # very first group consumes even h blocks first,
                            # matching their earlier arrival on the split
                            # sync/SWDGE queues
                            if seg == 0 and t == 0 and ci == 0:
                                hbs = list(range(0, HB, 2)) + \
                                    list(range(1, HB, 2))
                            else:
                                hbs = list(range(HB))
                            for k, hb in enumerate(hbs):
                                nc.tensor.matmul(pt[:], wt[c][:, hb, :],
                                                 ht[hb][:],
                                                 start=(k == 0),
                                                 stop=(k == HB - 1))
                            dst = acc_of(c)[:, t * TC_W:(t + 1) * TC_W]
                            if seg == 0:
                                nc.vector.tensor_copy(dst, pt[:])
                            else:
                                nc.vector.tensor_add(dst, dst, pt[:])
                            if last:
                                if c < Q_PER_CORE + KV_PER_CORE:
                                    rope_chunk(c, t)
                                if ci == 5:
                                    # V transposes for this chunk's 4 key
                                    # blocks; emitted a few psum groups after
                                    # the v drains so the DVE is surely ahead.
                                    for kv in range(KV_PER_CORE):
                                        for sb in range(4 * t, 4 * t + 4):
                                            ptt = ps2.tile(
                                                [128, 128], F16,
                                                name=f"pt2_{kv}_{sb}",
                                                tag="ps2")
                                            nc.tensor.transpose(
                                                ptt[:],
                                                accv[kv][:, sb * 128:
                                                         (sb + 1) * 128],
                                                ident[:],
                                            )
                                            vtile = vp.tile(
                                                [128, 128], BF16,
                                                name=f"v{kv}_{sb}",
                                                tag=f"v{kv}_{sb}")
                                            nc.scalar.copy(vtile[:], ptt[:])
                                            vnat[kv * SB + sb] = vtile

            # ---- stage 3 + 4 share the attention-output accumulator.
            # One tile per (head, chunk): keeps stage-4 LdWeights deps at
            # chunk granularity so stage 4 can start before the s3 tail.
            with tc.tile_pool(name="acco", bufs=1) as accop, \
                 tc.tile_pool(name="wop", bufs=8) as wops:
                acco = [[accop.tile([128, TC_W], BF16, name=f"acco{g}_{t}",
                                    tag=f"acco{g}_{t}") for t in range(TCH)]
                        for g in range(Q_PER_CORE)]

                def load_wo(n):
                    wq4 = []
                    for q4 in range(4):
                        wn = wops.tile([128, 2, TC_W], BF16,
                                       name=f"wo_{n}_{q4}", tag="wo")
                        nc.sync.dma_start(
                            out=wn[:],
                            in_=wo[q4 * 256:(q4 + 1) * 256,
                                   n * TC_W:(n + 1) * TC_W]
                            .rearrange("(g p) c -> p g c", p=128),
                        )
                        wq4.append(wn)
                    return wq4

                # prefetch the first two n-chunks' wo tiles BEFORE stage 3:
                # emitted later, their DMAs sit behind the stage-3 pool
                # barriers on the sync queue and stall stage 4's first groups
                wo_pre = {n: load_wo(n) for n in (0, 1)}

                # ---- stage 3: attention; batched emission, 1-batch pipeline
                with tc.tile_pool(name="probs", bufs=4) as pp, \
                     tc.tile_pool(name="recip", bufs=2) as rcp, \
                     tc.tile_pool(name="rbc", bufs=2) as rbp, \
                     tc.tile_pool(name="ps_s", bufs=2, space="PSUM") as ps_s, \
                     tc.tile_pool(name="ps_pv", bufs=2, space="PSUM") as ps_pv, \
                     tc.tile_pool(name="ps_sum", bufs=2, space="PSUM") as ps_sm:

                    # batches of 4 key blocks; PE emission per batch is
                    # [4 scores fp16] [4 pv bf16] [4 sum bf16] so psum-bank
                    # switches (and their per-group overhead) amortize.
                    # Diagonal batch LAST: its serialized gpsimd selects then
                    # hide under the (t+1) batches of pipeline cover. For the
                    # very last (head, chunk) there is no cover after it, so
                    # that one runs its diagonal batch FIRST instead.
                    def bb_order(g, t):
                        if (g, t) == (Q_PER_CORE - 1, TCH - 1):
                            return [t] + list(range(t))
                        return list(range(t + 1))
                    batches = [(g, t, bb, ei)
                               for g in range(Q_PER_CORE)
                               for t in range(TCH)
                               for ei, bb in enumerate(bb_order(g, t))]
                    prs = {}

                    def emit_batch(bi):
                        g, t, bb, _ei = batches[bi]
                        kv = g // GROUP
                        kt = acc[Q_PER_CORE + kv]
                        prl = []
                        for p in range(2):
                            # 2-bank psum pair: two score matmuls, ONE wide
                            # exp (halves the activation-engine overhead)
                            sc = ps_s.tile([128, 2, TC_W], F32,
                                           name=f"sc_{g}_{t}_{bb}_{p}",
                                           tag="s")
                            for i in range(2):
                                sb = 4 * bb + 2 * p + i
                                nc.tensor.matmul(
                                    sc[:, i, :],
                                    kt[:, sb * 128:(sb + 1) * 128],
                                    acc[g][:, t * TC_W:(t + 1) * TC_W],
                                    start=True, stop=True,
                                )
                            pr = pp.tile([128, 2, TC_W], BF16,
                                         name=f"pr_{g}_{t}_{bb}_{p}",
                                         tag="pr")
                            # softmax scale folded into the activation
                            nc.scalar.activation(
                                pr[:], sc[:], mybir.ActivationFunctionType.Exp,
                                scale=float(SCALE))
                            for i in range(2):
                                sb = 4 * bb + 2 * p + i
                                jd = sb - 4 * t
                                if jd >= 0:
                                    # zero where key > query; gpsimd keeps
                                    # the DVE queue free for recip/final-mul
                                    nc.gpsimd.affine_select(
                                        out=pr[:, i, :], in_=pr[:, i, :],
                                        compare_op=mybir.AluOpType.is_ge,
                                        fill=0.0, base=-128 * jd,
                                        pattern=[[1, TC_W]],
                                        channel_multiplier=-1,
                                    )
                                prl.append(pr[:, i, :])
                        prs[bi] = prl

                    emit_batch(0)
                    pv = None
                    sm = None
                    pending = {}  # bi -> (g, t, pv, rcb) final-mul to emit
                    for bi, (g, t, bb, ei) in enumerate(batches):
                        if bi in pending:
                            # delayed normalization mul: broadcast has landed
                            # by now, so this never blocks the DVE queue
                            pg, pt, ppv, prcb = pending.pop(bi)
                            nc.vector.tensor_mul(acco[pg][pt][:], ppv[:],
                                                 prcb[:])
                        if bi + 1 < len(batches):
                            emit_batch(bi + 1)
                        kv = g // GROUP
                        if ei == 0:
                            pv = ps_pv.tile([128, TC_W], F32,
                                            name=f"pv_{g}_{t}", tag="pv")
                            sm = ps_sm.tile([1, TC_W], F32,
                                            name=f"sm_{g}_{t}", tag="sum")
                        prl = prs.pop(bi)
                        for i in range(4):
                            sb = 4 * bb + i
                            first = ei == 0 and i == 0
                            lastm = ei == t and i == 3
                            nc.tensor.matmul(pv[:], vnat[kv * SB + sb][:],
                                             prl[i], start=first,
                                             stop=lastm)
                        for i in range(4):
                            sb = 4 * bb + i
                            first = ei == 0 and i == 0
                            lastm = ei == t and i == 3
                            nc.tensor.matmul(sm[:], ones_c[:], prl[i],
                                             start=first,
                                             stop=lastm)
                        if ei == t:
                            # normalization chain, fully off the PE
                            rc = rcp.tile([1, TC_W], F32,
                                          name=f"rc_{g}_{t}", tag="rc")
                            nc.vector.reciprocal_approx_fast(rc[:], sm[:])
                            rcb = rbp.tile([128, TC_W], F32,
                                           name=f"rcb_{g}_{t}", tag="rcb")
                            nc.gpsimd.partition_broadcast(rcb[:], rc[:])
                            pending[bi + 2] = (g, t, pv, rcb)
                    for pg, pt, ppv, prcb in pending.values():
                        nc.vector.tensor_mul(acco[pg][pt][:], ppv[:], prcb[:])

                # ---- stage 4: out[t, n] = sum_g attn_g^T @ wo_g  (bf16)
                with tc.tile_pool(name="outp", bufs=4) as op, \
                     tc.tile_pool(name="ps4", bufs=4, space="PSUM") as ps4:
                    for n in range(H // TC_W):
                        wq4 = wo_pre[n] if n in wo_pre else load_wo(n)
                        for tb in range(SB):
                            po = ps4.tile([128, TC_W], F32,
                                          name=f"po_{n}_{tb}", tag="po")
                            for g in range(Q_PER_CORE):
                                nc.tensor.matmul(
                                    po[:],
                                    acco[g][tb // 4][:, (tb % 4) * 128:
                                                     (tb % 4 + 1) * 128],
                                    wq4[g // 2][:, g % 2, :],
                                    start=(g == 0),
                                    stop=(g == Q_PER_CORE - 1),
                                )
                            ot = op.tile([128, TC_W], F32,
                                         name=f"ot_{n}_{tb}", tag="ot")
                            nc.scalar.copy(ot[:], po[:])
                            nc.gpsimd.dma_start(
                                out=out[tb * 128:(tb + 1) * 128,
                                        n * TC_W:(n + 1) * TC_W],
                                in_=ot[:],
                            )

    nc.compile()
    return nc


def _get_compiled():
    global _compiled
    if _compiled is None:
        _compiled = _build()
    return _compiled


_EVEN_ODD = np.concatenate([np.arange(0, HD, 2), np.arange(1, HD, 2)])


def _prep_core_inputs(hidden_states, positions, wqkv, wo):
    """Returns list of 8 in_maps (core c = 4*b + t)."""
    inv_freq = ROPE_BASE ** (-np.arange(0, HD, 2, dtype=np.float32) / HD)
    ident = np.eye(128, dtype=np.float16)
    ones_c = np.ones((128, 1), dtype=ml_dtypes.bfloat16)

    per_batch = []
    for b in range(B):
        hid_t = np.ascontiguousarray(hidden_states[b].T.astype(np.float16))
        ang = positions[b].astype(np.float32)[:, None] * inv_freq[None, :]
        cos = np.cos(ang).T.astype(np.float32)  # [64, S]
        sin = np.sin(ang).T.astype(np.float32)
        cos2 = np.ascontiguousarray(
            np.concatenate([cos, cos], axis=0).astype(np.float16))
        sinpm = np.ascontiguousarray(
            np.concatenate([-sin, sin], axis=0).astype(np.float16))
        per_batch.append((hid_t, cos2, sinpm))

    in_maps = []
    for c in range(N_CORES):
        b, t = c // TP, c % TP
        hid_t, cos2, sinpm = per_batch[b]
        blocks = []
        for gh in range(Q_PER_CORE):  # q heads, permuted (softmax scale is
            h = Q_PER_CORE * t + gh   # applied in the exp activation)
            blocks.append(wqkv[:, h * HD:(h + 1) * HD][:, _EVEN_ODD])
        for m in range(KV_PER_CORE):  # k heads, permuted
            h = KV_PER_CORE * t + m
            blocks.append(
                wqkv[:, NH * HD + h * HD: NH * HD + (h + 1) * HD][:, _EVEN_ODD])
        for m in range(KV_PER_CORE):  # v heads, natural
            h = KV_PER_CORE * t + m
            base = (NH + NKV) * HD
            blocks.append(wqkv[:, base + h * HD: base + (h + 1) * HD])
        w12 = np.ascontiguousarray(
            np.concatenate(blocks, axis=1).astype(np.float16))
        wo_shard = np.ascontiguousarray(
            wo[Q_PER_CORE * HD * t: Q_PER_CORE * HD * (t + 1), :]
            .astype(ml_dtypes.bfloat16))
        in_maps.append({
            "hid_t": hid_t, "w12": w12, "wo": wo_shard,
            "cos2": cos2, "sinpm": sinpm,
            "identd": ident, "onescd": ones_c,
        })
    return in_maps


def kernel(hidden_states, positions, wqkv, wo):
    hidden_states = np.asarray(hidden_states)
    positions = np.asarray(positions)
    wqkv = np.asarray(wqkv)
    wo = np.asarray(wo)
    nc = _get_compiled()
    in_maps = _prep_core_inputs(hidden_states, positions, wqkv, wo)
    res = run_bass_kernel_spmd(nc, in_maps, list(range(N_CORES)))
    full = np.zeros((B, S, H), dtype=np.float32)
    for c in range(N_CORES):
        full[c // TP] += res.results[c]["out"]
    return full
